# revision 6
# baseline (speedup 1.0000x reference)
"""Trainium2 Bass kernel for the pairwise+triplewise cycle-consistency loss.

Strategy (8 NeuronCores, tensor-parallel over rows of each [N,N] block):
  - All six cycle-term matrices have the form  A = U @ nf_j^T  with
    U = nf_i (pairs) or U = nf_i @ G_k (triples), G_k = nf_k^T nf_k [D,D].
    This collapses the [N,N]@[N,N] triple products into [D,D] Gram matmuls.
  - Each core owns a 512-row block R_c. Per term it computes A[R_c,:] and
    A^T[R_c,:] directly (f32r matmuls), row-softmaxes both, AllGathers the
    normalized S21_hat (bf16), and computes M^T column-tiles
    (S12_hat @ S21_hat)^T[jtile, R_c] with bf16 matmuls, accumulating
    rowmax/colmax/diag stats on the fly. The scalar loss is assembled on host.
  - Grams are computed as per-core partials and AllReduced (overlapped with
    the three pair terms, which do not need them).
"""
import sys
sys.path.insert(0, "/opt/trn_rl_repo")

import math
import numpy as np

import concourse.bass as bass
import concourse.mybir as mybir
import concourse.tile as tile
from concourse import bacc
from concourse.bass_utils import run_bass_kernel_spmd
from concourse.masks import make_identity

F32 = mybir.dt.float32
F32R = mybir.dt.float32r
BF16 = mybir.dt.bfloat16
AX = mybir.AxisListType
OP = mybir.AluOpType
ACT = mybir.ActivationFunctionType

NTOK = 4096          # rows per view
D = 1024             # feature dim
NC = 8               # cores
RPC = NTOK // NC     # rows per core (512)
P = 128
NRT = RPC // P       # rowtiles per core (4)
NS = 8               # 512-col strips of A
DKB = D // P         # d-blocks (8)
NKB = NTOK // P      # k-blocks of the M product (32)
NJG = 8              # jtile groups (4 jtiles each) in M product
SCALE = math.log(NTOK) / 0.1
MARGIN = 0.5

# term table: (is_tri, gram_idx, lhsA, rhsA, lhsAT, rhsAT); lhs names index x_i,
# rhs names index f_i. For tri terms lhs is G[gram_idx] @ x_i.
TERMS = [
    (False, None, 0, 1, 1, 0),   # S01
    (False, None, 0, 2, 2, 0),   # S02
    (False, None, 1, 2, 2, 1),   # S12
    (True, 2, 0, 1, 1, 0),       # S02 @ S21 = nf0 G2 nf1^T
    (True, 1, 0, 2, 2, 0),       # S01 @ S12 = nf0 G1 nf2^T
    (True, 0, 1, 2, 2, 1),       # S10 @ S02 = nf1 G0 nf2^T
]

OUT_W = RPC + 32 + NRT   # racc 512 | colmax32 32 | diag 4


def build_program():
    nc = bacc.Bacc("TRN2", target_bir_lowering=False, debug=False, num_devices=NC)

    xs = [nc.dram_tensor(f"x{i}", [D, RPC], F32R, kind="ExternalInput") for i in range(3)]
    ws = [nc.dram_tensor(f"w{i}", [RPC, D], F32R, kind="ExternalInput") for i in range(3)]
    fs = [nc.dram_tensor(f"f{i}", [D, NTOK], F32R, kind="ExternalInput") for i in range(3)]
    wsel_in = nc.dram_tensor("wsel", [P, P], F32, kind="ExternalInput")
    out = nc.dram_tensor("out", [6, P, OUT_W], F32, kind="ExternalOutput")

    with tile.TileContext(nc) as tc:
        with (
            tc.tile_pool(name="cst", bufs=1) as cst,
            tc.tile_pool(name="lhs", bufs=2) as lhsp,
            tc.tile_pool(name="rhs", bufs=2) as rhsp,
            tc.tile_pool(name="aq", bufs=4) as aqp,
            tc.tile_pool(name="pt", bufs=1) as ptp,
            tc.tile_pool(name="qbf", bufs=1) as qbfp,
            tc.tile_pool(name="qsb", bufs=3) as qsbp,
            tc.tile_pool(name="st", bufs=2) as stp,
            tc.tile_pool(name="sm", bufs=4) as smp,
            tc.tile_pool(name="psA", bufs=2, space="PSUM") as psA,
            tc.tile_pool(name="psT", bufs=2, space="PSUM") as psT,
            tc.tile_pool(name="psM", bufs=4, space="PSUM") as psM,
            tc.tile_pool(name="dram", bufs=1, space="DRAM") as dram,
            tc.tile_pool(name="dram2", bufs=2, space="DRAM") as dram2,
        ):
            # constants
            ident = cst.tile([P, P], F32)
            make_identity(nc, ident)
            wsel = cst.tile([P, P], F32)
            nc.sync.dma_start(wsel[:], wsel_in[:])
            nwsel = cst.tile([P, P], F32)
            nc.vector.tensor_scalar_mul(nwsel[:], wsel[:], -1.0)
            # imask4[p, 128b+p] = 1 for b in 0..3 (diag candidate positions)
            imask4 = cst.tile([P, NRT, P], F32)
            for b in range(NRT):
                nc.vector.tensor_copy(imask4[:, b, :], ident[:])

            # ---------------- Gram phase ----------------
            gin = dram.tile([3 * D, D], F32, tag="gin")
            gout = dram.tile([3 * D, D], F32, tag="gout", addr_space="Shared")
            for k in range(3):
                w_sb = aqp.tile([P, NRT, D], F32R, tag="aq", name=f"w_sb{k}")
                nc.sync.dma_start(w_sb[:], ws[k].rearrange("(o p) d -> p o d", p=P))
                for d1 in range(DKB):
                    for d2 in range(2):
                        ps = psA.tile([P, 512], F32, tag="psA", name=f"gps{k}_{d1}_{d2}")
                        for nt in range(NRT):
                            nc.tensor.matmul(
                                ps[:], w_sb[:, nt, d1 * P:(d1 + 1) * P],
                                w_sb[:, nt, d2 * 512:(d2 + 1) * 512],
                                start=(nt == 0), stop=(nt == NRT - 1))
                        gtmp = stp.tile([P, 512], F32, tag="gtmp", name=f"gt{k}_{d1}_{d2}")
                        nc.scalar.copy(gtmp[:], ps[:])
                        nc.sync.dma_start(
                            gin[k * D + d1 * P: k * D + (d1 + 1) * P,
                                d2 * 512:(d2 + 1) * 512], gtmp[:])
            nc.gpsimd.collective_compute(
                "AllReduce", OP.add, replica_groups=[list(range(NC))],
                ins=[gin[:]], outs=[gout[:]])

            # ---------------- helpers ----------------
            def load_x(i, nm):
                t = lhsp.tile([P, DKB, RPC], F32R, tag="lhs", name=f"x_{nm}")
                nc.sync.dma_start(t[:], xs[i].rearrange("(o p) r -> p o r", p=P))
                return t

            def compute_ut(gk, i, nm):
                """U^T[:, R_c] = G_k @ x_i  -> [128, DKB, RPC] f32r tile."""
                x_sb = load_x(i, f"utx_{nm}")
                ut = lhsp.tile([P, DKB, RPC], F32R, tag="lhs", name=f"ut_{nm}")
                for grp in range(2):
                    pss = [psM.tile([P, 512], F32, tag="psM", name=f"utps_{nm}_{grp}_{d4}")
                           for d4 in range(4)]
                    for half in range(2):
                        gh = rhsp.tile([P, 4, D], F32R, tag="rhs", name=f"gh_{nm}_{grp}_{half}")
                        nc.sync.dma_start(
                            gh[:], gout[gk * D + half * 512: gk * D + (half + 1) * 512]
                            .rearrange("(o p) d -> p o d", p=P).bitcast(F32R))
                        for d4 in range(4):
                            dp = 4 * grp + d4
                            for db in range(4):
                                nc.tensor.matmul(
                                    pss[d4][:], gh[:, db, dp * P:(dp + 1) * P],
                                    x_sb[:, 4 * half + db, :],
                                    start=(half == 0 and db == 0),
                                    stop=(half == 1 and db == 3))
                    for d4 in range(4):
                        nc.scalar.copy(ut[:, 4 * grp + d4, :], pss[d4][:])
                return ut

            def side_chunk(lhs_t, fj, nm):
                """A-side chunk [RPC, 4096] raw logits (pre-scale) in 4 quarter tiles."""
                chunk = [aqp.tile([P, NTOK], F32, tag="aq", name=f"ch_{nm}_{rt}")
                         for rt in range(NRT)]
                for s in range(NS):
                    rsb = rhsp.tile([P, DKB, 512], F32R, tag="rhs", name=f"rs_{nm}_{s}")
                    nc.sync.dma_start(
                        rsb[:], fs[fj][:, s * 512:(s + 1) * 512]
                        .rearrange("(o p) n -> p o n", p=P))
                    for rt in range(NRT):
                        ps = psA.tile([P, 512], F32, tag="psA", name=f"aps_{nm}_{s}_{rt}")
                        for kb in range(DKB):
                            nc.tensor.matmul(
                                ps[:], lhs_t[:, kb, rt * P:(rt + 1) * P],
                                rsb[:, kb, :], start=(kb == 0), stop=(kb == DKB - 1))
                        nc.scalar.copy(chunk[rt][:, s * 512:(s + 1) * 512], ps[:])
                return chunk

            def softmax_quarter(q, nm):
                """in-place exp(SCALE*(x - rowmax)); returns reciprocal row sum."""
                rm = smp.tile([P, 1], F32, tag="sm", name=f"rm_{nm}")
                nc.vector.reduce_max(rm[:], q[:], axis=AX.X)
                bias = smp.tile([P, 1], F32, tag="sm", name=f"bias_{nm}")
                nc.vector.tensor_scalar_mul(bias[:], rm[:], -SCALE)
                ssum = smp.tile([P, 1], F32, tag="sm", name=f"ss_{nm}")
                nc.scalar.activation(q[:], q[:], ACT.Exp, bias=bias[:], scale=SCALE,
                                     accum_out=ssum[:])
                rs = smp.tile([P, 1], F32, tag="sm", name=f"rs_{nm}")
                nc.vector.reciprocal(rs[:], ssum[:])
                return rs

            def at_phase(t, lhs_t, fj):
                """A^T side: softmax rows -> normalized bf16 -> allgather kick."""
                ag_in = dram2.tile([RPC, NTOK], BF16, tag="agin", name=f"agin{t}")
                ag_out = dram2.tile([NTOK, NTOK], BF16, tag="agout",
                                    addr_space="Shared", name=f"agout{t}")
                chunk = side_chunk(lhs_t, fj, f"at{t}")
                for rt in range(NRT):
                    rq = softmax_quarter(chunk[rt], f"at{t}_{rt}")
                    qb = qbfp.tile([P, NTOK], BF16, tag="qbf", name=f"qb{t}_{rt}")
                    nc.scalar.activation(qb[:], chunk[rt][:], ACT.Copy, bias=0.0,
                                         scale=rq[:])
                    nc.sync.dma_start(ag_in[rt * P:(rt + 1) * P, :], qb[:])
                nc.gpsimd.collective_compute(
                    "AllGather", OP.bypass, replica_groups=[list(range(NC))],
                    ins=[ag_in[:]], outs=[ag_out[:]])
                return ag_out

            def a_phase(t, lhs_t, fj):
                """A side: softmax, normalize, transpose into bf16 PT tile."""
                pt = ptp.tile([P, NKB, RPC], BF16, tag="pt", name=f"pt{t}")
                chunk = side_chunk(lhs_t, fj, f"a{t}")
                for rt in range(NRT):
                    rp = softmax_quarter(chunk[rt], f"a{t}_{rt}")
                    nc.scalar.activation(chunk[rt][:], chunk[rt][:], ACT.Copy,
                                         bias=0.0, scale=rp[:])
                    for j in range(NKB):
                        tp = psT.tile([P, P], F32, tag="psT", name=f"tp{t}_{rt}_{j}")
                        nc.tensor.transpose(tp[:], chunk[rt][:, j * P:(j + 1) * P],
                                            ident[:])
                        nc.vector.tensor_copy(pt[:, j, rt * P:(rt + 1) * P], tp[:])
                return pt

            def m_phase(u, pt, ag_out):
                """M^T tiles = (S12_hat @ S21_hat)^T[jtile, R_c]; stats to out[u].

                Diag handling per jtile j (all basic DVE ops):
                  tmp4 = msb * imask4           (candidate diag at (p, 128b+p))
                  dv4[p,b] = rowsum of block b  (= msb[p,128b+p])
                  dv4w = dv4 * (-w[j,b])        (w one-hot over (j,b), host input)
                  dvallw[:, :, j] = dv4w        (for the diag output)
                  msb += imask4 * bcast(dv4w)   (zeroes the true diag only)
                """
                racc = stp.tile([P, RPC], F32, tag="racc", name=f"racc{u}")
                nc.vector.memset(racc[:], 0.0)
                dvallw = stp.tile([P, NRT, 32], F32, tag="dvall", name=f"dvall{u}")
                cm32 = stp.tile([P, 32], F32, tag="cm32", name=f"cm32{u}")
                nwselJ = nwsel.rearrange("p (j b) -> p j b", b=NRT)
                for jg in range(NJG):
                    pss = [psM.tile([P, 512], F32, tag="psM", name=f"mps{u}_{jg}_{j2}")
                           for j2 in range(4)]
                    for kb in range(NKB):
                        qsb = qsbp.tile([P, 512], BF16, tag="qsb",
                                        name=f"qs{u}_{jg}_{kb}")
                        nc.sync.dma_start(
                            qsb[:], ag_out[kb * P:(kb + 1) * P,
                                           jg * 512:(jg + 1) * 512])
                        for j2 in range(4):
                            nc.tensor.matmul(
                                pss[j2][:], qsb[:, j2 * P:(j2 + 1) * P], pt[:, kb, :],
                                start=(kb == 0), stop=(kb == NKB - 1))
                    for j2 in range(4):
                        j = 4 * jg + j2
                        msb = stp.tile([P, 512], F32, tag="msb", name=f"msb{u}_{j}")
                        nc.scalar.copy(msb[:], pss[j2][:])
                        tmp4 = stp.tile([P, 512], F32, tag="tmp4", name=f"t4_{u}_{j}")
                        nc.vector.tensor_tensor(tmp4[:], msb[:], imask4[:], op=OP.mult)
                        dv4 = smp.tile([P, NRT], F32, tag="sm4", name=f"dv4_{u}_{j}")
                        nc.vector.reduce_sum(
                            dv4[:], tmp4.rearrange("p (b q) -> p b q", q=P), axis=AX.X)
                        dv4w = smp.tile([P, NRT], F32, tag="sm4", name=f"dvw_{u}_{j}")
                        nc.vector.tensor_tensor(dv4w[:], dv4[:], nwselJ[:, j, :],
                                                op=OP.mult)
                        nc.vector.tensor_copy(dvallw[:, :, j], dv4w[:])
                        sc = stp.tile([P, NRT, P], F32, tag="tmp4", name=f"sc_{u}_{j}")
                        nc.vector.tensor_tensor(
                            sc[:], imask4[:],
                            dv4w[:, :, None].to_broadcast((P, NRT, P)), op=OP.mult)
                        nc.vector.tensor_add(
                            msb.rearrange("p (b q) -> p b q", q=P), msb.rearrange(
                                "p (b q) -> p b q", q=P), sc[:])
                        nc.vector.reduce_max(cm32[:, j:j + 1], msb[:], axis=AX.X)
                        nc.vector.tensor_tensor(racc[:], racc[:], msb[:], op=OP.max)
                # diag output = -sum_j dvallw
                dsum = smp.tile([P, NRT], F32, tag="sm4", name=f"dsum{u}")
                nc.vector.reduce_sum(dsum[:], dvallw[:], axis=AX.X)
                diag = smp.tile([P, NRT], F32, tag="sm4", name=f"diag{u}")
                nc.vector.tensor_scalar_mul(diag[:], dsum[:], -1.0)
                nc.sync.dma_start(out[u, :, 0:RPC], racc[:])
                nc.sync.dma_start(out[u, :, RPC:RPC + 32], cm32[:])
                nc.sync.dma_start(out[u, :, RPC + 32:OUT_W], diag[:])

            # ---------------- main pipeline ----------------
            prev = None   # (u, pt, ag_out)
            for t, (is_tri, gk, la, ra, lat, rat) in enumerate(TERMS):
                if is_tri:
                    lhs_at = compute_ut(gk, lat, f"at{t}")
                else:
                    lhs_at = load_x(lat, f"at{t}")
                ag_out = at_phase(t, lhs_at, rat)
                if is_tri:
                    lhs_a = compute_ut(gk, la, f"a{t}")
                else:
                    lhs_a = load_x(la, f"a{t}")
                if prev is not None:
                    m_phase(*prev)
                pt = a_phase(t, lhs_a, ra)
                prev = (t, pt, ag_out)
            m_phase(*prev)

    nc.finalize()
    return nc


_PROGRAM = None


def _get_program():
    global _PROGRAM
    if _PROGRAM is None:
        _PROGRAM = build_program()
    return _PROGRAM


def _normalize(x):
    n = np.linalg.norm(x.astype(np.float32), axis=-1, keepdims=True)
    return (x / np.maximum(n, 1e-12)).astype(np.float32)


def _build_in_maps(inputs):
    nf = [_normalize(np.asarray(inputs[k], np.float32))
          for k in ("feat0", "feat1", "feat2")]
    nfT = [np.ascontiguousarray(x.T) for x in nf]

    in_maps = []
    for c in range(NC):
        rows = slice(c * RPC, (c + 1) * RPC)
        m = {}
        for i in range(3):
            m[f"x{i}"] = np.ascontiguousarray(nfT[i][:, rows])
            m[f"w{i}"] = np.ascontiguousarray(nf[i][rows])
            m[f"f{i}"] = nfT[i]
        wsel = np.zeros((P, P), np.float32)
        for b in range(NRT):
            j = 4 * c + b
            wsel[:, 4 * j + b] = 1.0     # wselJ[p, j, b] layout
        m["wsel"] = wsel
        in_maps.append(m)
    return in_maps


def kernel(feat0, feat1, feat2):
    in_maps = _build_in_maps({"feat0": feat0, "feat1": feat1, "feat2": feat2})
    nc = _get_program()
    res = run_bass_kernel_spmd(nc, in_maps, core_ids=list(range(NC)))

    # host-side reduction
    L = np.zeros(6, np.float64)
    for t in range(6):
        rowpart = 0.0
        colmax = np.full(NTOK, -np.inf)
        diag_g = np.zeros(NTOK)
        for c in range(NC):
            o = res.results[c]["out"][t].astype(np.float64)
            racc = o[:, 0:RPC]
            cm32 = o[:, RPC:RPC + 32]
            dacc = o[:, RPC + 32:OUT_W]
            rowmax_local = racc.max(axis=0)                   # [512]
            diag_local = dacc.T.reshape(RPC)                  # [512]
            rowpart += np.maximum(rowmax_local + MARGIN - diag_local, 0.0).sum()
            colmax = np.maximum(colmax, cm32.T.reshape(NTOK))
            diag_g[c * RPC:(c + 1) * RPC] = diag_local
        colpart = np.maximum(colmax + MARGIN - diag_g, 0.0).sum()
        L[t] = (rowpart + colpart) / (2.0 * NTOK)
    loss = (L[0] + L[1] + L[2]) / 3.0 + (L[3] + L[4] + L[5]) / 3.0
    return np.float32(loss)


if __name__ == "__main__":
    rng = np.random.default_rng(0)
    f0 = rng.standard_normal((NTOK, D), dtype=np.float32)
    f1 = rng.standard_normal((NTOK, D), dtype=np.float32)
    f2 = rng.standard_normal((NTOK, D), dtype=np.float32)
    print("loss:", kernel(f0, f1, f2))


# revision 7
# speedup vs baseline: 6608.4582x; 6608.4582x over previous
"""Trainium2 Bass kernel for the pairwise+triplewise cycle-consistency loss.

Strategy (8 NeuronCores, tensor-parallel over rows of each [N,N] block):
  - All six cycle-term matrices have the form  A = U @ nf_j^T  with
    U = nf_i (pairs) or U = nf_i @ G_k (triples), G_k = nf_k^T nf_k [D,D].
    This collapses the [N,N]@[N,N] triple products into [D,D] Gram matmuls.
  - Each core owns a 512-row block R_c. Per term it computes A[R_c,:] and
    A^T[R_c,:] directly (f32r matmuls), row-softmaxes both, AllGathers the
    normalized S21_hat (bf16), and computes M^T column-tiles
    (S12_hat @ S21_hat)^T[jtile, R_c] with bf16 matmuls, accumulating
    rowmax/colmax/diag stats on the fly. The scalar loss is assembled on host.
  - Grams are computed as per-core partials and AllReduced (overlapped with
    the three pair terms, which do not need them).
"""
import sys
sys.path.insert(0, "/opt/trn_rl_repo")

import math
import numpy as np

import concourse.bass as bass
import concourse.mybir as mybir
import concourse.tile as tile
from concourse import bacc
from concourse.bass_utils import run_bass_kernel_spmd
from concourse.masks import make_identity

F32 = mybir.dt.float32
F32R = mybir.dt.float32r
BF16 = mybir.dt.bfloat16
AX = mybir.AxisListType
OP = mybir.AluOpType
ACT = mybir.ActivationFunctionType

NTOK = 4096          # rows per view
D = 1024             # feature dim
NC = 8               # cores
RPC = NTOK // NC     # rows per core (512)
P = 128
NRT = RPC // P       # rowtiles per core (4)
NS = 8               # 512-col strips of A
DKB = D // P         # d-blocks (8)
NKB = NTOK // P      # k-blocks of the M product (32)
NJG = 8              # jtile groups (4 jtiles each) in M product
SCALE = math.log(NTOK) / 0.1
MARGIN = 0.5

# term table: (is_tri, gram_idx, lhsA, rhsA, lhsAT, rhsAT); lhs names index x_i,
# rhs names index f_i. For tri terms lhs is G[gram_idx] @ x_i.
TERMS = [
    (False, None, 0, 1, 1, 0),   # S01
    (False, None, 0, 2, 2, 0),   # S02
    (False, None, 1, 2, 2, 1),   # S12
    (True, 2, 0, 1, 1, 0),       # S02 @ S21 = nf0 G2 nf1^T
    (True, 1, 0, 2, 2, 0),       # S01 @ S12 = nf0 G1 nf2^T
    (True, 0, 1, 2, 2, 1),       # S10 @ S02 = nf1 G0 nf2^T
]

OUT_W = RPC + 32 + NRT   # racc 512 | colmax32 32 | diag 4


def build_program():
    nc = bacc.Bacc("TRN2", target_bir_lowering=False, debug=False, num_devices=NC)

    xs = [nc.dram_tensor(f"x{i}", [D, RPC], F32R, kind="ExternalInput") for i in range(3)]
    ws = [nc.dram_tensor(f"w{i}", [RPC, D], F32R, kind="ExternalInput") for i in range(3)]
    fs = [nc.dram_tensor(f"f{i}", [D, NTOK], F32R, kind="ExternalInput") for i in range(3)]
    wsel_in = nc.dram_tensor("wsel", [P, P], F32, kind="ExternalInput")
    out = nc.dram_tensor("out", [6, P, OUT_W], F32, kind="ExternalOutput")

    with tile.TileContext(nc) as tc:
        with (
            tc.tile_pool(name="cst", bufs=1) as cst,
            tc.tile_pool(name="lhs", bufs=2) as lhsp,
            tc.tile_pool(name="rhs", bufs=2) as rhsp,
            tc.tile_pool(name="aq", bufs=4) as aqp,
            tc.tile_pool(name="pt", bufs=1) as ptp,
            tc.tile_pool(name="qbf", bufs=1) as qbfp,
            tc.tile_pool(name="qsb", bufs=3) as qsbp,
            tc.tile_pool(name="st", bufs=2) as stp,
            tc.tile_pool(name="sm", bufs=4) as smp,
            tc.tile_pool(name="psA", bufs=2, space="PSUM") as psA,
            tc.tile_pool(name="psT", bufs=2, space="PSUM") as psT,
            tc.tile_pool(name="psM", bufs=4, space="PSUM") as psM,
            tc.tile_pool(name="dram", bufs=1, space="DRAM") as dram,
            tc.tile_pool(name="dram2", bufs=2, space="DRAM") as dram2,
        ):
            # constants
            ident = cst.tile([P, P], F32)
            make_identity(nc, ident)
            wsel = cst.tile([P, P], F32)
            nc.sync.dma_start(wsel[:], wsel_in[:])
            nwsel = cst.tile([P, P], F32)
            nc.vector.tensor_scalar_mul(nwsel[:], wsel[:], -1.0)
            # imask4[p, 128b+p] = 1 for b in 0..3 (diag candidate positions)
            imask4 = cst.tile([P, NRT, P], F32)
            for b in range(NRT):
                nc.vector.tensor_copy(imask4[:, b, :], ident[:])

            # ---------------- Gram phase ----------------
            gin = dram.tile([3 * D, D], F32, tag="gin")
            gout = dram.tile([3 * D, D], F32, tag="gout", addr_space="Shared")
            for k in range(3):
                w_sb = aqp.tile([P, NRT, D], F32R, tag="aq", name=f"w_sb{k}")
                nc.sync.dma_start(w_sb[:], ws[k].rearrange("(o p) d -> p o d", p=P))
                for d1 in range(DKB):
                    for d2 in range(2):
                        ps = psA.tile([P, 512], F32, tag="psA", name=f"gps{k}_{d1}_{d2}")
                        for nt in range(NRT):
                            nc.tensor.matmul(
                                ps[:], w_sb[:, nt, d1 * P:(d1 + 1) * P],
                                w_sb[:, nt, d2 * 512:(d2 + 1) * 512],
                                start=(nt == 0), stop=(nt == NRT - 1))
                        gtmp = stp.tile([P, 512], F32, tag="gtmp", name=f"gt{k}_{d1}_{d2}")
                        nc.scalar.copy(gtmp[:], ps[:])
                        nc.sync.dma_start(
                            gin[k * D + d1 * P: k * D + (d1 + 1) * P,
                                d2 * 512:(d2 + 1) * 512], gtmp[:])
            nc.gpsimd.collective_compute(
                "AllReduce", OP.add, replica_groups=[list(range(NC))],
                ins=[gin[:]], outs=[gout[:]])

            # ---------------- helpers ----------------
            def load_x(i, nm):
                t = lhsp.tile([P, DKB, RPC], F32R, tag="lhs", name=f"x_{nm}")
                nc.sync.dma_start(t[:], xs[i].rearrange("(o p) r -> p o r", p=P))
                return t

            def compute_ut(gk, i, nm):
                """U^T[:, R_c] = G_k @ x_i  -> [128, DKB, RPC] f32r tile."""
                x_sb = load_x(i, f"utx_{nm}")
                ut = lhsp.tile([P, DKB, RPC], F32R, tag="lhs", name=f"ut_{nm}")
                for grp in range(2):
                    pss = [psM.tile([P, 512], F32, tag="psM", name=f"utps_{nm}_{grp}_{d4}")
                           for d4 in range(4)]
                    for half in range(2):
                        gh = rhsp.tile([P, 4, D], F32R, tag="rhs", name=f"gh_{nm}_{grp}_{half}")
                        nc.sync.dma_start(
                            gh[:], gout[gk * D + half * 512: gk * D + (half + 1) * 512]
                            .rearrange("(o p) d -> p o d", p=P).bitcast(F32R))
                        for d4 in range(4):
                            dp = 4 * grp + d4
                            for db in range(4):
                                nc.tensor.matmul(
                                    pss[d4][:], gh[:, db, dp * P:(dp + 1) * P],
                                    x_sb[:, 4 * half + db, :],
                                    start=(half == 0 and db == 0),
                                    stop=(half == 1 and db == 3))
                    for d4 in range(4):
                        nc.scalar.copy(ut[:, 4 * grp + d4, :], pss[d4][:])
                return ut

            def side_chunk(lhs_t, fj, nm):
                """A-side chunk [RPC, 4096] raw logits (pre-scale) in 4 quarter tiles."""
                chunk = [aqp.tile([P, NTOK], F32, tag="aq", name=f"ch_{nm}_{rt}")
                         for rt in range(NRT)]
                for s in range(NS):
                    rsb = rhsp.tile([P, DKB, 512], F32R, tag="rhs", name=f"rs_{nm}_{s}")
                    nc.sync.dma_start(
                        rsb[:], fs[fj][:, s * 512:(s + 1) * 512]
                        .rearrange("(o p) n -> p o n", p=P))
                    for rt in range(NRT):
                        ps = psA.tile([P, 512], F32, tag="psA", name=f"aps_{nm}_{s}_{rt}")
                        for kb in range(DKB):
                            nc.tensor.matmul(
                                ps[:], lhs_t[:, kb, rt * P:(rt + 1) * P],
                                rsb[:, kb, :], start=(kb == 0), stop=(kb == DKB - 1))
                        nc.scalar.copy(chunk[rt][:, s * 512:(s + 1) * 512], ps[:])
                return chunk

            def softmax_quarter(q, nm):
                """in-place exp(SCALE*(x - rowmax)); returns reciprocal row sum."""
                rm = smp.tile([P, 1], F32, tag="sm", name=f"rm_{nm}")
                nc.vector.reduce_max(rm[:], q[:], axis=AX.X)
                bias = smp.tile([P, 1], F32, tag="sm", name=f"bias_{nm}")
                nc.vector.tensor_scalar_mul(bias[:], rm[:], -SCALE)
                ssum = smp.tile([P, 1], F32, tag="sm", name=f"ss_{nm}")
                nc.scalar.activation(q[:], q[:], ACT.Exp, bias=bias[:], scale=SCALE,
                                     accum_out=ssum[:])
                rs = smp.tile([P, 1], F32, tag="sm", name=f"rs_{nm}")
                nc.vector.reciprocal(rs[:], ssum[:])
                return rs

            def at_phase(t, lhs_t, fj):
                """A^T side: softmax rows -> normalized bf16 -> allgather kick."""
                ag_in = dram2.tile([RPC, NTOK], BF16, tag="agin", name=f"agin{t}")
                ag_out = dram2.tile([NTOK, NTOK], BF16, tag="agout",
                                    addr_space="Shared", name=f"agout{t}")
                chunk = side_chunk(lhs_t, fj, f"at{t}")
                for rt in range(NRT):
                    rq = softmax_quarter(chunk[rt], f"at{t}_{rt}")
                    qb = qbfp.tile([P, NTOK], BF16, tag="qbf", name=f"qb{t}_{rt}")
                    nc.scalar.activation(qb[:], chunk[rt][:], ACT.Copy, bias=0.0,
                                         scale=rq[:])
                    nc.sync.dma_start(ag_in[rt * P:(rt + 1) * P, :], qb[:])
                nc.gpsimd.collective_compute(
                    "AllGather", OP.bypass, replica_groups=[list(range(NC))],
                    ins=[ag_in[:]], outs=[ag_out[:]])
                return ag_out

            def a_phase(t, lhs_t, fj):
                """A side: softmax, normalize, transpose into bf16 PT tile."""
                pt = ptp.tile([P, NKB, RPC], BF16, tag="pt", name=f"pt{t}")
                chunk = side_chunk(lhs_t, fj, f"a{t}")
                for rt in range(NRT):
                    rp = softmax_quarter(chunk[rt], f"a{t}_{rt}")
                    nc.scalar.activation(chunk[rt][:], chunk[rt][:], ACT.Copy,
                                         bias=0.0, scale=rp[:])
                    for j in range(NKB):
                        tp = psT.tile([P, P], F32, tag="psT", name=f"tp{t}_{rt}_{j}")
                        nc.tensor.transpose(tp[:], chunk[rt][:, j * P:(j + 1) * P],
                                            ident[:])
                        nc.vector.tensor_copy(pt[:, j, rt * P:(rt + 1) * P], tp[:])
                return pt

            def m_phase(u, pt, ag_out):
                """M^T tiles = (S12_hat @ S21_hat)^T[jtile, R_c]; stats to out[u].

                Diag handling per jtile j (all basic DVE ops):
                  tmp4 = msb * imask4           (candidate diag at (p, 128b+p))
                  dv4[p,b] = rowsum of block b  (= msb[p,128b+p])
                  dv4w = dv4 * (-w[j,b])        (w one-hot over (j,b), host input)
                  dvallw[:, :, j] = dv4w        (for the diag output)
                  msb += imask4 * bcast(dv4w)   (zeroes the true diag only)
                """
                racc = stp.tile([P, RPC], F32, tag="racc", name=f"racc{u}")
                nc.vector.memset(racc[:], 0.0)
                dvallw = stp.tile([P, NRT, 32], F32, tag="dvall", name=f"dvall{u}")
                cm32 = stp.tile([P, 32], F32, tag="cm32", name=f"cm32{u}")
                nwselJ = nwsel.rearrange("p (j b) -> p j b", b=NRT)
                for jg in range(NJG):
                    pss = [psM.tile([P, 512], F32, tag="psM", name=f"mps{u}_{jg}_{j2}")
                           for j2 in range(4)]
                    for kb in range(NKB):
                        qsb = qsbp.tile([P, 512], BF16, tag="qsb",
                                        name=f"qs{u}_{jg}_{kb}")
                        nc.sync.dma_start(
                            qsb[:], ag_out[kb * P:(kb + 1) * P,
                                           jg * 512:(jg + 1) * 512])
                        for j2 in range(4):
                            nc.tensor.matmul(
                                pss[j2][:], qsb[:, j2 * P:(j2 + 1) * P], pt[:, kb, :],
                                start=(kb == 0), stop=(kb == NKB - 1))
                    for j2 in range(4):
                        j = 4 * jg + j2
                        msb = stp.tile([P, 512], F32, tag="msb", name=f"msb{u}_{j}")
                        nc.scalar.copy(msb[:], pss[j2][:])
                        tmp4 = stp.tile([P, 512], F32, tag="tmp4", name=f"t4_{u}_{j}")
                        nc.vector.tensor_tensor(tmp4[:], msb[:], imask4[:], op=OP.mult)
                        dv4 = smp.tile([P, NRT], F32, tag="sm4", name=f"dv4_{u}_{j}")
                        nc.vector.reduce_sum(
                            dv4[:], tmp4.rearrange("p (b q) -> p b q", q=P), axis=AX.X)
                        dv4w = smp.tile([P, NRT], F32, tag="sm4", name=f"dvw_{u}_{j}")
                        nc.vector.tensor_tensor(dv4w[:], dv4[:], nwselJ[:, j, :],
                                                op=OP.mult)
                        nc.vector.tensor_copy(dvallw[:, :, j], dv4w[:])
                        sc = stp.tile([P, NRT, P], F32, tag="tmp4", name=f"sc_{u}_{j}")
                        nc.vector.tensor_tensor(
                            sc[:], imask4[:],
                            dv4w[:, :, None].to_broadcast((P, NRT, P)), op=OP.mult)
                        nc.vector.tensor_add(
                            msb.rearrange("p (b q) -> p b q", q=P), msb.rearrange(
                                "p (b q) -> p b q", q=P), sc[:])
                        nc.vector.reduce_max(cm32[:, j:j + 1], msb[:], axis=AX.X)
                        nc.vector.tensor_tensor(racc[:], racc[:], msb[:], op=OP.max)
                # diag output = -sum_j dvallw
                dsum = smp.tile([P, NRT], F32, tag="sm4", name=f"dsum{u}")
                nc.vector.reduce_sum(dsum[:], dvallw[:], axis=AX.X)
                diag = smp.tile([P, NRT], F32, tag="sm4", name=f"diag{u}")
                nc.vector.tensor_scalar_mul(diag[:], dsum[:], -1.0)
                nc.sync.dma_start(out[u, :, 0:RPC], racc[:])
                nc.sync.dma_start(out[u, :, RPC:RPC + 32], cm32[:])
                nc.sync.dma_start(out[u, :, RPC + 32:OUT_W], diag[:])

            # ---------------- main pipeline ----------------
            prev = None   # (u, pt, ag_out)
            for t, (is_tri, gk, la, ra, lat, rat) in enumerate(TERMS):
                if is_tri:
                    lhs_at = compute_ut(gk, lat, f"at{t}")
                else:
                    lhs_at = load_x(lat, f"at{t}")
                ag_out = at_phase(t, lhs_at, rat)
                if is_tri:
                    lhs_a = compute_ut(gk, la, f"a{t}")
                else:
                    lhs_a = load_x(la, f"a{t}")
                if prev is not None:
                    m_phase(*prev)
                pt = a_phase(t, lhs_a, ra)
                prev = (t, pt, ag_out)
            m_phase(*prev)

    nc.finalize()
    return nc


_PROGRAM = None


def _get_program():
    global _PROGRAM
    if _PROGRAM is None:
        _PROGRAM = build_program()
    return _PROGRAM


def _normalize(x):
    n = np.linalg.norm(x.astype(np.float32), axis=-1, keepdims=True)
    return (x / np.maximum(n, 1e-12)).astype(np.float32)


def _build_in_maps(inputs):
    nf = [_normalize(np.asarray(inputs[k], np.float32))
          for k in ("feat0", "feat1", "feat2")]
    nfT = [np.ascontiguousarray(x.T) for x in nf]

    in_maps = []
    for c in range(NC):
        rows = slice(c * RPC, (c + 1) * RPC)
        m = {}
        for i in range(3):
            m[f"x{i}"] = np.ascontiguousarray(nfT[i][:, rows])
            m[f"w{i}"] = np.ascontiguousarray(nf[i][rows])
            m[f"f{i}"] = nfT[i]
        wsel = np.zeros((P, P), np.float32)
        for b in range(NRT):
            j = 4 * c + b
            wsel[:, 4 * j + b] = 1.0     # wselJ[p, j, b] layout
        m["wsel"] = wsel
        in_maps.append(m)
    return in_maps


def _reduce(results):
    """results: list (per core) of {'out': [6, 128, OUT_W]} -> scalar loss."""
    L = np.zeros(6, np.float64)
    for t in range(6):
        rowpart = 0.0
        colmax = np.full(NTOK, -np.inf)
        diag_g = np.zeros(NTOK)
        for c in range(NC):
            o = results[c]["out"][t].astype(np.float64)
            racc = o[:, 0:RPC]
            cm32 = o[:, RPC:RPC + 32]
            dacc = o[:, RPC + 32:OUT_W]
            rowmax_local = racc.max(axis=0)                   # [512]
            diag_local = dacc.T.reshape(RPC)                  # [512]
            rowpart += np.maximum(rowmax_local + MARGIN - diag_local, 0.0).sum()
            colmax = np.maximum(colmax, cm32.T.reshape(NTOK))
            diag_g[c * RPC:(c + 1) * RPC] = diag_local
        colpart = np.maximum(colmax + MARGIN - diag_g, 0.0).sum()
        L[t] = (rowpart + colpart) / (2.0 * NTOK)
    loss = (L[0] + L[1] + L[2]) / 3.0 + (L[3] + L[4] + L[5]) / 3.0
    return np.float32(loss)


def kernel(feat0, feat1, feat2):
    in_maps = _build_in_maps({"feat0": feat0, "feat1": feat1, "feat2": feat2})
    nc = _get_program()
    res = run_bass_kernel_spmd(nc, in_maps, core_ids=list(range(NC)))
    return _reduce(res.results)


if __name__ == "__main__":
    rng = np.random.default_rng(0)
    f0 = rng.standard_normal((NTOK, D), dtype=np.float32)
    f1 = rng.standard_normal((NTOK, D), dtype=np.float32)
    f2 = rng.standard_normal((NTOK, D), dtype=np.float32)
    print("loss:", kernel(f0, f1, f2))
